# revision 26
# baseline (speedup 1.0000x reference)
"""GatedGCN message-passing layer on 8 Trainium2 NeuronCores.

Strategy (dst-sharded, aggregate-first):
  - Node rows (dsts) sharded across 8 cores (6250 each).
  - agg0 = sum_e a_e * X[col_e]  computed on-device: X table (bf16, 256B rows)
    lives in HBM; edges gathered with dma_gather; per-edge a-scale on DVE;
    segment-sum via identity-matmul PSUM accumulation over degree-sorted
    128-dst windows (one edge slot per dst lane per group).
  - agg = agg0 @ Wn + (sum_e a_e) * bn   (linearity: fold Wn after aggregation)
  - gate = sigmoid(X@Wgi + bgi + agg@Wgn + bgn); out = gate*(agg-X)+X.
  - dma_gather indices are int16 -> X table split in two 25000-row halves; the
    hi-half aggregate is computed in its own degree-sorted order, bounced
    through an HBM scratch and re-gathered into lo-order for the merge.
"""

import sys
import types

sys.path.insert(0, "/opt/trn_rl_repo")

import numpy as np
import ml_dtypes

BF16 = ml_dtypes.bfloat16

N = 50000
E = 800000
D = 96
CORES = 8
DPC = N // CORES  # dsts per core
HALF = N // 2  # table split for int16 indices
LANES = 128
WB = 5  # windows per matmul batch (5*97=485 <= 512 psum free)
NW = (DPC + LANES - 1) // LANES  # 49 windows per pass
NQ = 4  # SWDGE queues in use
GCH = 3  # groups per gather chunk
DSTS = NW * LANES  # 6272 padded dst slots per core


def _install_ntff_hook():
    try:
        import antenv.axon_hooks  # noqa: F401

        return
    except ImportError:
        pass
    try:
        from trn_agent_boot.trn_boot import _ntff_profile_via_ctypes

        hook = _ntff_profile_via_ctypes("/opt/axon/libaxon_pjrt.so")
        mod = types.ModuleType("antenv.axon_hooks")
        mod.get_axon_ntff_profile_hook = lambda: hook
        mod.set_axon_ntff_profile_hook = lambda h: None
        sys.modules["antenv.axon_hooks"] = mod
    except Exception:
        pass


# ---------------------------------------------------------------- host prep


def _prep_pass(ldst, tloc, av, nw):
    """Per-pass structures for one core.

    ldst: local dst id per edge; tloc: table-local src row; av: a value.
    Returns dict with sigma (dst order), deg (per local dst), and per-dst
    sorted edge arrays (tloc/av ordered by dst, with offsets).
    """
    dsts = nw * LANES
    deg = np.bincount(ldst, minlength=dsts).astype(np.int64)
    order = np.argsort(ldst, kind="stable")
    offs = np.zeros(dsts + 1, np.int64)
    np.cumsum(deg, out=offs[1:])
    sigma = np.argsort(-deg, kind="stable")
    return dict(
        deg=deg,
        offs=offs,
        tloc_sorted=tloc[order],
        av_sorted=av[order],
        sigma=sigma,
        wmax=deg[sigma[::LANES]].copy(),  # max degree per window
    )


def _batches(nw, wmax_shared):
    """Static batch structure shared by all cores: list of (w0, nwb, Db)."""
    out = []
    for b in range((nw + WB - 1) // WB):
        w0 = b * WB
        nwb = min(WB, nw - w0)
        Db = int(wmax_shared[w0:w0 + nwb].max()) if nwb else 0
        out.append((w0, nwb, Db))
    return out


def _fill_pass(p, batches, slot_off, idx_off, gidx16, a2, nw):
    """Fill gather-index and a-value arrays for one pass of one core."""
    sigma, deg, offs = p["sigma"], p["deg"], p["offs"]
    tls, avs = p["tloc_sorted"], p["av_sorted"]
    s = slot_off
    for (w0, nwb, Db) in batches:
        if Db == 0:
            continue
        # dstm[g, wi, l] = local dst at window w0+wi lane l
        lanes = sigma[w0 * LANES:(w0 + nwb) * LANES].reshape(nwb, LANES)
        dstm = np.broadcast_to(lanes[None, :, :], (Db, nwb, LANES))
        g = np.arange(Db)[:, None, None]
        valid = g < deg[dstm]
        eidx = offs[dstm] + np.minimum(g, np.maximum(deg[dstm] - 1, 0))
        eidx = np.minimum(eidx, max(len(tls) - 1, 0))
        iv = np.where(valid, tls[eidx], 0).astype(np.int16)
        avv = np.where(valid, avs[eidx], 0.0).astype(np.float32)
        nslots = Db * nwb
        # position n = (slot-local)*128 + lane ; slot-local = g*nwb+wi
        ivf = iv.reshape(nslots * LANES)
        avf = avv.reshape(nslots, LANES)
        # index layout: position n -> [n%16, idx_off + n//16]
        c0 = idx_off + s * (LANES // 16) - slot_off * (LANES // 16)
        cols = nslots * LANES // 16
        gidx16[:, c0:c0 + cols] = ivf.reshape(cols, 16).T
        a2[:, 2 * s:2 * (s + nslots):2] = avf.T
        a2[:, 2 * s + 1:2 * (s + nslots) + 1:2] = avf.T
        s += nslots
    return s


def prep(X, a_vals, row, col):
    """Returns (meta, per_core_inputs, host_ctx)."""
    row = np.asarray(row).astype(np.int64)
    col = np.asarray(col).astype(np.int64)
    av = np.asarray(a_vals, np.float32)
    X = np.asarray(X, np.float32)

    # shared bf16 X table, 128-col rows, col 96 = 1.0 (for sum-of-a column)
    xtab = np.zeros((N, 128), BF16)
    xtab[:, :96] = X.astype(BF16)
    xtab[:, 96] = BF16(1.0)

    core = row // DPC
    passes = []  # per core: (lo_pass, hi_pass)
    for k in range(CORES):
        m = core == k
        rk, ck, ak = row[m] - k * DPC, col[m], av[m]
        mlo = ck < HALF
        lo = _prep_pass(rk[mlo], ck[mlo], ak[mlo], NW)
        hi = _prep_pass(rk[~mlo], ck[~mlo] - HALF, ak[~mlo], NW)
        passes.append((lo, hi))

    # shared (max over cores) window-degree profiles -> static batch structure
    wmax_lo = np.max([p[0]["wmax"] for p in passes], axis=0)
    wmax_hi = np.max([p[1]["wmax"] for p in passes], axis=0)
    b_lo = _batches(NW, wmax_lo)
    b_hi = _batches(NW, wmax_hi)
    s_hi = sum(d * n for (_, n, d) in b_hi)
    s_lo = sum(d * n for (_, n, d) in b_lo)
    n_perm = DSTS  # one gathered row per lo-ordered dst slot
    idx_cols = (s_hi + s_lo) * (LANES // 16) + n_perm // 16
    meta = dict(b_lo=b_lo, b_hi=b_hi, s_hi=s_hi, s_lo=s_lo,
                idx_cols=idx_cols, a2_cols=2 * (s_hi + s_lo))

    per_core = []
    sig_lo_all = []
    for k in range(CORES):
        lo, hi = passes[k]
        gidx16 = np.zeros((16, idx_cols), np.int16)
        a2 = np.zeros((128, meta["a2_cols"]), BF16)
        nh = _fill_pass(hi, b_hi, 0, 0, gidx16, a2, NW)
        assert nh == s_hi
        nl = _fill_pass(lo, b_lo, s_hi, s_hi * (LANES // 16), gidx16, a2, NW)
        assert nl == s_hi + s_lo
        # perm indices: lo-order position j -> hi-order position of same dst
        pos_hi = np.empty(DSTS, np.int64)
        pos_hi[hi["sigma"]] = np.arange(DSTS)
        pidx = pos_hi[lo["sigma"]].astype(np.int16)
        c0 = (s_hi + s_lo) * (LANES // 16)
        gidx16[:, c0:] = pidx.reshape(DSTS // 16, 16).T
        gidx = np.tile(gidx16, (8, 1))

        # per-core node features, lo-order, feature-major, with ones row
        gids = np.minimum(k * DPC + lo["sigma"], N - 1)
        realm = lo["sigma"] < DPC
        xs = np.where(realm[:, None], X[gids], 0.0)
        xt1s = np.zeros((97, DSTS), BF16)
        xt1s[:96] = xs.T.astype(BF16)
        xt1s[96] = np.where(realm, 1.0, 0.0).astype(BF16)
        per_core.append(dict(xtab=xtab, gidx=gidx, a2=a2, xt1s=xt1s))
        sig_lo_all.append(lo["sigma"])

    return meta, per_core, sig_lo_all


# ---------------------------------------------------------------- bass build


def build(meta, phase=99):
    import concourse.bacc as bacc
    import concourse.mybir as mybir
    import concourse.tile as tile

    bf = mybir.dt.bfloat16
    f32 = mybir.dt.float32
    nc = bacc.Bacc(None, target_bir_lowering=False, num_swdge_queues=4)

    xtab = nc.dram_tensor("xtab", [N, 128], bf, kind="ExternalInput")
    gidx = nc.dram_tensor("gidx", [128, meta["idx_cols"]], mybir.dt.int16,
                          kind="ExternalInput")
    a2d = nc.dram_tensor("a2", [128, meta["a2_cols"]], bf, kind="ExternalInput")
    xt1d = nc.dram_tensor("xt1s", [97, DSTS], bf, kind="ExternalInput")
    wn1 = nc.dram_tensor("wn1", [97, 96], bf, kind="ExternalInput")
    wgi1 = nc.dram_tensor("wgi1", [97, 96], bf, kind="ExternalInput")
    wgnD = nc.dram_tensor("wgn", [96, 96], bf, kind="ExternalInput")
    outd = nc.dram_tensor("out", [96, DSTS], f32, kind="ExternalOutput")
    hscr = nc.dram_tensor("hscr", [DSTS, 128], bf, kind="Internal")

    identD = nc.inline_tensor(np.eye(128, dtype=BF16), "ident")

    IW = LANES // 16  # idx cols per slot

    def emit(tc, cpool, mpool, fpool, bpool, ps_e, ps_t, ps_f):
        gi = bpool.tile([128, meta["idx_cols"]], mybir.dt.int16)
        nc.sync.dma_start(out=gi[:], in_=gidx[:, :])
        ident = cpool.tile([128, 128], bf)
        nc.sync.dma_start(out=ident[:], in_=identD[:, :])
        a2 = bpool.tile([128, meta["a2_cols"]], bf)
        nc.sync.dma_start(out=a2[:], in_=a2d[:, :])
        wn1t = cpool.tile([97, 96], bf)
        nc.sync.dma_start(out=wn1t[:], in_=wn1[:, :])
        wgi1t = cpool.tile([97, 96], bf)
        nc.sync.dma_start(out=wgi1t[:], in_=wgi1[:, :])
        wgnt = cpool.tile([96, 96], bf)
        nc.sync.dma_start(out=wgnt[:], in_=wgnD[:, :])
        xt1 = bpool.tile([97, DSTS], bf)
        nc.sync.dma_start(out=xt1[:], in_=xt1d[:, :])

        def finish_dummy():
            outsb = bpool.tile([96, DSTS], f32)
            nc.vector.memset(outsb[:], 0.0)
            nc.sync.dma_start(out=outd[:, :], in_=outsb[:])

        swdge_ctr = [0]

        def next_q():
            q = swdge_ctr[0] % NQ
            swdge_ctr[0] += 1
            return q

        def gather_scale(batch_i, slot_off, idx_off, nwb, Db, src_ap):
            """Chunked gather + a-scale; yields (msgs_tile, g0, ng)."""
            out = []
            g0 = 0
            while g0 < Db:
                ng = min(GCH, Db - g0)
                nslots = ng * nwb
                ni = nslots * LANES
                so = slot_off + g0 * nwb
                io = idx_off + g0 * nwb * IW
                msgs = mpool.tile([128, nslots, 128], bf, tag="msgs")
                nc.gpsimd.dma_gather(
                    msgs[:],
                    src_ap,
                    gi[:, io:io + nslots * IW],
                    ni,
                    ni,
                    128,
                    queue_num=next_q(),
                    single_packet=False,
                )
                if phase >= 3:
                    m4 = msgs[:].rearrange("p s (c t) -> p s c t", t=2)[:, :, :49, :]
                    a4 = (
                        a2[:, 2 * so:2 * (so + nslots)]
                        .rearrange("p (s t) -> p s t", t=2)
                        .unsqueeze(2)
                        .broadcast_to((128, nslots, 49, 2))
                    )
                    nc.vector.tensor_tensor(m4, m4, a4, mybir.AluOpType.mult)
                out.append((msgs, g0, ng))
                g0 += ng
            return out

        if phase <= 1:
            finish_dummy()
            return
        if phase <= 3:
            (w0, nwb, Db) = meta["b_hi"][0]
            msgs = gather_scale(0, 0, 0, nwb, Db, xtab[HALF:, :])
            fl = fpool.tile([128, nwb, 128], bf, tag="fl")
            nc.vector.tensor_copy(fl[:], msgs[:, :nwb, :])
            dst = hscr[:nwb * LANES, :].rearrange("(w p) c -> p w c", p=LANES)
            nc.sync.dma_start(out=dst, in_=fl[:])
            finish_dummy()
            return

        # ---- hi pass: aggregate in hi-order, bounce via HBM scratch
        slot_off, idx_off, bi = 0, 0, 0
        for (w0, nwb, Db) in meta["b_hi"]:
            fl = fpool.tile([128, nwb, 128], bf, tag="fl")
            if Db == 0:
                nc.vector.memset(fl[:], 0.0)
            else:
                chunks = gather_scale(bi, slot_off, idx_off, nwb, Db,
                                      xtab[HALF:, :])
                ps = ps_e.tile([128, nwb, 97], f32, tag="pse")
                for (msgs, g0, ng) in chunks:
                    for g in range(ng):
                        nc.tensor.matmul(
                            ps[:], ident[:],
                            msgs[:, g * nwb:(g + 1) * nwb, :97],
                            start=(g0 + g == 0), stop=(g0 + g == Db - 1))
                nc.vector.memset(fl[:, :, 97:], 0.0)
                nc.scalar.activation(fl[:, :, :97], ps[:],
                                     mybir.ActivationFunctionType.Copy)
            dst = hscr[w0 * LANES:(w0 + nwb) * LANES, :].rearrange(
                "(w p) c -> p w c", p=LANES)
            nc.sync.dma_start(out=dst, in_=fl[:])
            slot_off += Db * nwb
            idx_off += Db * nwb * IW
            bi += 1

        if phase <= 4:
            finish_dummy()
            return

        # ---- lo pass: aggregate into persistent agg0lo (lane-major)
        agg0T = bpool.tile([97, DSTS], bf)
        agg0lo = bpool.tile([128, NW, 128], bf)
        for (w0, nwb, Db) in meta["b_lo"]:
            dstsl = agg0lo[:, w0:w0 + nwb, :]
            if Db == 0:
                nc.vector.memset(dstsl, 0.0)
            else:
                ps = ps_e.tile([128, nwb, 97], f32, tag="pse")
                chunks = gather_scale(bi, slot_off, idx_off, nwb, Db,
                                      xtab[:HALF, :])
                for (msgs, g0, ng) in chunks:
                    for g in range(ng):
                        nc.tensor.matmul(
                            ps[:], ident[:],
                            msgs[:, g * nwb:(g + 1) * nwb, :97],
                            start=(g0 + g == 0), stop=(g0 + g == Db - 1))
                nc.scalar.activation(dstsl[:, :, :97], ps[:],
                                     mybir.ActivationFunctionType.Copy)
            slot_off += Db * nwb
            idx_off += Db * nwb * IW
            bi += 1

        # ---- merge permuted hi aggregate, transpose windows into agg0T
        perm_idx0 = (meta["s_hi"] + meta["s_lo"]) * IW
        for (w0, nwb, Db) in meta["b_lo"]:
            permt = mpool.tile([128, nwb, 128], bf, tag="perm")
            ni = nwb * LANES
            nc.gpsimd.dma_gather(
                permt[:],
                hscr[:, :],
                gi[:, perm_idx0 + w0 * (LANES // 16):
                   perm_idx0 + (w0 + nwb) * (LANES // 16)],
                ni,
                ni,
                128,
                queue_num=next_q(),
                single_packet=False,
            )
            mg = agg0lo[:, w0:w0 + nwb, :97]
            nc.vector.tensor_tensor(mg, mg, permt[:, :, :97],
                                    mybir.AluOpType.add)
            for wi in range(nwb):
                tp = ps_t.tile([97, 128], bf, tag="tp")
                nc.tensor.transpose(
                    tp[:], agg0lo[:, w0 + wi, :97], ident[:])
                nc.vector.tensor_copy(
                    agg0T[:, (w0 + wi) * LANES:(w0 + wi + 1) * LANES],
                    tp[:])

        if phase <= 5:
            finish_dummy()
            return

        # ---- fold Wn/bn, gate, combine + store (feature-major 512 tiles)
        aggT = bpool.tile([96, DSTS], bf)
        c0 = 0
        while c0 < DSTS:
            w = min(512, DSTS - c0)
            fp = ps_f.tile([96, w], f32, tag="fg")
            nc.tensor.matmul(fp[:], wn1t[:], agg0T[:, c0:c0 + w],
                             start=True, stop=True)
            nc.scalar.activation(aggT[:, c0:c0 + w], fp[:],
                                 mybir.ActivationFunctionType.Copy)
            gp = ps_f.tile([96, w], f32, tag="fg")
            nc.tensor.matmul(gp[:], wgi1t[:], xt1[:, c0:c0 + w],
                             start=True, stop=False)
            nc.tensor.matmul(gp[:], wgnt[:], aggT[:, c0:c0 + w],
                             start=False, stop=True)
            gate = fpool.tile([96, 512], bf, tag="gate")
            nc.scalar.activation(gate[:, :w], gp[:],
                                 mybir.ActivationFunctionType.Sigmoid)
            diff = fpool.tile([96, 512], bf, tag="diff")
            nc.vector.tensor_tensor(diff[:, :w], aggT[:, c0:c0 + w],
                                    xt1[:96, c0:c0 + w],
                                    mybir.AluOpType.subtract)
            nc.vector.tensor_tensor(diff[:, :w], diff[:, :w], gate[:, :w],
                                    mybir.AluOpType.mult)
            outsb = fpool.tile([96, 512], f32, tag="outsb")
            nc.vector.tensor_tensor(outsb[:, :w], diff[:, :w],
                                    xt1[:96, c0:c0 + w],
                                    mybir.AluOpType.add)
            nc.sync.dma_start(out=outd[:, c0:c0 + w], in_=outsb[:, :w])
            c0 += w

    with tile.TileContext(nc) as tc:
        with (
            tc.tile_pool(name="const", bufs=1) as cpool,
            tc.tile_pool(name="msgs", bufs=14) as mpool,
            tc.tile_pool(name="flush", bufs=4) as fpool,
            tc.tile_pool(name="big", bufs=1) as bpool,
            tc.tile_pool(name="ps_edge", bufs=5, space="PSUM") as ps_e,
            tc.tile_pool(name="ps_tp", bufs=1, space="PSUM") as ps_t,
            tc.tile_pool(name="ps_fg", bufs=2, space="PSUM") as ps_f,
        ):
            emit(tc, cpool, mpool, fpool, bpool, ps_e, ps_t, ps_f)

    nc.compile()

    # Align each gather's SWDGE queue with its Tile-assigned DMASW lane so a
    # semaphore is only ever updated from one queue (ucode shadow-sem rule).
    for ins in nc.inst_map.values():
        if isinstance(ins, mybir.InstDMAGatherAnt):
            si = ins.sync_info
            if si and si.on_update:
                lane = int(si.on_update[0].ant_name.split("_")[0][5:])
                ins.queue_num = lane % NQ
    return nc


# ---------------------------------------------------------------- entrypoint

_CACHE = {}


def kernel(X, a_vals, Wn, bn, Wgi, bgi, Wgn, bgn, row, col):
    _install_ntff_hook()
    from concourse.bass_utils import run_bass_kernel_spmd

    meta, per_core, sig_lo = prep(X, a_vals, row, col)
    key = (str(meta["b_lo"]), str(meta["b_hi"]))
    if key not in _CACHE:
        _CACHE[key] = build(meta)
    nc = _CACHE[key]

    Wn, bn, Wgi, bgi, Wgn, bgn = (np.asarray(w, np.float32)
                                  for w in (Wn, bn, Wgi, bgi, Wgn, bgn))
    wts = dict(
        wn1=np.concatenate([Wn, bn[None]], 0).astype(BF16),
        wgi1=np.concatenate([Wgi, (bgi + bgn)[None]], 0).astype(BF16),
        wgn=Wgn.astype(BF16),
    )
    in_maps = [dict(c, **wts) for c in per_core]
    res = run_bass_kernel_spmd(nc, in_maps, core_ids=list(range(CORES)),
                               trace=kernel._trace)
    kernel._last = res

    out = np.empty((N, 96), np.float32)
    for k in range(CORES):
        o = res.results[k]["out"]  # [96, DSTS]
        sig = sig_lo[k]
        realm = sig < DPC
        out[k * DPC + sig[realm]] = o[:, realm].T
    return out


kernel._trace = False
kernel._last = None


# revision 27
# speedup vs baseline: 1.0429x; 1.0429x over previous
"""GatedGCN message-passing layer on 8 Trainium2 NeuronCores.

Strategy (dst-sharded, aggregate-first):
  - Node rows (dsts) sharded across 8 cores (6250 each).
  - agg0 = sum_e a_e * X[col_e]  computed on-device: X table (bf16, 256B rows)
    lives in HBM; edges gathered with dma_gather; per-edge a-scale on DVE;
    segment-sum via identity-matmul PSUM accumulation over degree-sorted
    128-dst windows (one edge slot per dst lane per group).
  - agg = agg0 @ Wn + (sum_e a_e) * bn   (linearity: fold Wn after aggregation)
  - gate = sigmoid(X@Wgi + bgi + agg@Wgn + bgn); out = gate*(agg-X)+X.
  - dma_gather indices are int16 -> X table split in two 25000-row halves; the
    hi-half aggregate is computed in its own degree-sorted order, bounced
    through an HBM scratch and re-gathered into lo-order for the merge.
"""

import sys
import types

sys.path.insert(0, "/opt/trn_rl_repo")

import numpy as np
import ml_dtypes

BF16 = ml_dtypes.bfloat16

N = 50000
E = 800000
D = 96
CORES = 8
DPC = N // CORES  # dsts per core
HALF = N // 2  # table split for int16 indices
LANES = 128
WB = 5  # windows per matmul batch (5*97=485 <= 512 psum free)
NW = (DPC + LANES - 1) // LANES  # 49 windows per pass
NQ = 4  # SWDGE queues in use
GCH = 3  # groups per gather chunk
DSTS = NW * LANES  # 6272 padded dst slots per core


def _install_ntff_hook():
    try:
        import antenv.axon_hooks  # noqa: F401

        return
    except ImportError:
        pass
    try:
        from trn_agent_boot.trn_boot import _ntff_profile_via_ctypes

        hook = _ntff_profile_via_ctypes("/opt/axon/libaxon_pjrt.so")
        mod = types.ModuleType("antenv.axon_hooks")
        mod.get_axon_ntff_profile_hook = lambda: hook
        mod.set_axon_ntff_profile_hook = lambda h: None
        sys.modules["antenv.axon_hooks"] = mod
    except Exception:
        pass


# ---------------------------------------------------------------- host prep


def _prep_pass(ldst, tloc, av, nw):
    """Per-pass structures for one core.

    ldst: local dst id per edge; tloc: table-local src row; av: a value.
    Returns dict with sigma (dst order), deg (per local dst), and per-dst
    sorted edge arrays (tloc/av ordered by dst, with offsets).
    """
    dsts = nw * LANES
    deg = np.bincount(ldst, minlength=dsts).astype(np.int64)
    order = np.argsort(ldst, kind="stable")
    offs = np.zeros(dsts + 1, np.int64)
    np.cumsum(deg, out=offs[1:])
    sigma = np.argsort(-deg, kind="stable")
    return dict(
        deg=deg,
        offs=offs,
        tloc_sorted=tloc[order],
        av_sorted=av[order],
        sigma=sigma,
        wmax=deg[sigma[::LANES]].copy(),  # max degree per window
    )


def _batches(nw, wmax_shared):
    """Static batch structure shared by all cores: list of (w0, nwb, Db)."""
    out = []
    for b in range((nw + WB - 1) // WB):
        w0 = b * WB
        nwb = min(WB, nw - w0)
        Db = int(wmax_shared[w0:w0 + nwb].max()) if nwb else 0
        out.append((w0, nwb, Db))
    return out


def _fill_pass(p, batches, slot_off, idx_off, gidx16, a2, nw):
    """Fill gather-index and a-value arrays for one pass of one core."""
    sigma, deg, offs = p["sigma"], p["deg"], p["offs"]
    tls, avs = p["tloc_sorted"], p["av_sorted"]
    s = slot_off
    for (w0, nwb, Db) in batches:
        if Db == 0:
            continue
        # dstm[g, wi, l] = local dst at window w0+wi lane l
        lanes = sigma[w0 * LANES:(w0 + nwb) * LANES].reshape(nwb, LANES)
        dstm = np.broadcast_to(lanes[None, :, :], (Db, nwb, LANES))
        g = np.arange(Db)[:, None, None]
        valid = g < deg[dstm]
        eidx = offs[dstm] + np.minimum(g, np.maximum(deg[dstm] - 1, 0))
        eidx = np.minimum(eidx, max(len(tls) - 1, 0))
        iv = np.where(valid, tls[eidx], 0).astype(np.int16)
        avv = np.where(valid, avs[eidx], 0.0).astype(np.float32)
        nslots = Db * nwb
        # position n = (slot-local)*128 + lane ; slot-local = g*nwb+wi
        ivf = iv.reshape(nslots * LANES)
        avf = avv.reshape(nslots, LANES)
        # index layout: position n -> [n%16, idx_off + n//16]
        c0 = idx_off + s * (LANES // 16) - slot_off * (LANES // 16)
        cols = nslots * LANES // 16
        gidx16[:, c0:c0 + cols] = ivf.reshape(cols, 16).T
        a2[:, 2 * s:2 * (s + nslots):2] = avf.T
        a2[:, 2 * s + 1:2 * (s + nslots) + 1:2] = avf.T
        s += nslots
    return s


def prep(X, a_vals, row, col):
    """Returns (meta, per_core_inputs, host_ctx)."""
    row = np.asarray(row).astype(np.int64)
    col = np.asarray(col).astype(np.int64)
    av = np.asarray(a_vals, np.float32)
    X = np.asarray(X, np.float32)

    # shared bf16 X table, 128-col rows, col 96 = 1.0 (for sum-of-a column)
    xtab = np.zeros((N, 128), BF16)
    xtab[:, :96] = X.astype(BF16)
    xtab[:, 96] = BF16(1.0)

    core = row // DPC
    passes = []  # per core: (lo_pass, hi_pass)
    for k in range(CORES):
        m = core == k
        rk, ck, ak = row[m] - k * DPC, col[m], av[m]
        mlo = ck < HALF
        lo = _prep_pass(rk[mlo], ck[mlo], ak[mlo], NW)
        hi = _prep_pass(rk[~mlo], ck[~mlo] - HALF, ak[~mlo], NW)
        passes.append((lo, hi))

    # shared (max over cores) window-degree profiles -> static batch structure
    wmax_lo = np.max([p[0]["wmax"] for p in passes], axis=0)
    wmax_hi = np.max([p[1]["wmax"] for p in passes], axis=0)
    b_lo = _batches(NW, wmax_lo)
    b_hi = _batches(NW, wmax_hi)
    s_hi = sum(d * n for (_, n, d) in b_hi)
    s_lo = sum(d * n for (_, n, d) in b_lo)
    n_perm = DSTS  # one gathered row per lo-ordered dst slot
    idx_cols = (s_hi + s_lo) * (LANES // 16) + n_perm // 16
    meta = dict(b_lo=b_lo, b_hi=b_hi, s_hi=s_hi, s_lo=s_lo,
                idx_cols=idx_cols, a2_cols=2 * (s_hi + s_lo))

    per_core = []
    sig_lo_all = []
    for k in range(CORES):
        lo, hi = passes[k]
        gidx16 = np.zeros((16, idx_cols), np.int16)
        a2 = np.zeros((128, meta["a2_cols"]), BF16)
        nh = _fill_pass(hi, b_hi, 0, 0, gidx16, a2, NW)
        assert nh == s_hi
        nl = _fill_pass(lo, b_lo, s_hi, s_hi * (LANES // 16), gidx16, a2, NW)
        assert nl == s_hi + s_lo
        # perm indices: lo-order position j -> hi-order position of same dst
        pos_hi = np.empty(DSTS, np.int64)
        pos_hi[hi["sigma"]] = np.arange(DSTS)
        pidx = pos_hi[lo["sigma"]].astype(np.int16)
        c0 = (s_hi + s_lo) * (LANES // 16)
        gidx16[:, c0:] = pidx.reshape(DSTS // 16, 16).T
        gidx = np.tile(gidx16, (8, 1))

        # per-core node features, lo-order, feature-major, with ones row
        gids = np.minimum(k * DPC + lo["sigma"], N - 1)
        realm = lo["sigma"] < DPC
        xs = np.where(realm[:, None], X[gids], 0.0)
        xt1s = np.zeros((97, DSTS), BF16)
        xt1s[:96] = xs.T.astype(BF16)
        xt1s[96] = np.where(realm, 1.0, 0.0).astype(BF16)
        per_core.append(dict(xtab=xtab, gidx=gidx, a2=a2, xt1s=xt1s))
        sig_lo_all.append(lo["sigma"])

    return meta, per_core, sig_lo_all


# ---------------------------------------------------------------- bass build


def build(meta, phase=99):
    import concourse.bacc as bacc
    import concourse.mybir as mybir
    import concourse.tile as tile

    bf = mybir.dt.bfloat16
    f32 = mybir.dt.float32
    nc = bacc.Bacc(None, target_bir_lowering=False, num_swdge_queues=4)

    xtab = nc.dram_tensor("xtab", [N, 128], bf, kind="ExternalInput")
    gidx = nc.dram_tensor("gidx", [128, meta["idx_cols"]], mybir.dt.int16,
                          kind="ExternalInput")
    a2d = nc.dram_tensor("a2", [128, meta["a2_cols"]], bf, kind="ExternalInput")
    xt1d = nc.dram_tensor("xt1s", [97, DSTS], bf, kind="ExternalInput")
    wn1 = nc.dram_tensor("wn1", [97, 96], bf, kind="ExternalInput")
    wgi1 = nc.dram_tensor("wgi1", [97, 96], bf, kind="ExternalInput")
    wgnD = nc.dram_tensor("wgn", [96, 96], bf, kind="ExternalInput")
    outd = nc.dram_tensor("out", [96, DSTS], f32, kind="ExternalOutput")
    hscr = nc.dram_tensor("hscr", [DSTS, 128], bf, kind="Internal")

    identD = nc.inline_tensor(np.eye(128, dtype=BF16), "ident")

    IW = LANES // 16  # idx cols per slot

    def emit(tc, cpool, mpool, fpool, bpool, ps_e, ps_t, ps_f):
        gi = bpool.tile([128, meta["idx_cols"]], mybir.dt.int16)
        nc.sync.dma_start(out=gi[:], in_=gidx[:, :])
        ident = cpool.tile([128, 128], bf)
        nc.sync.dma_start(out=ident[:], in_=identD[:, :])
        a2 = bpool.tile([128, meta["a2_cols"]], bf)
        nc.sync.dma_start(out=a2[:], in_=a2d[:, :])
        wn1t = cpool.tile([97, 96], bf)
        nc.sync.dma_start(out=wn1t[:], in_=wn1[:, :])
        wgi1t = cpool.tile([97, 96], bf)
        nc.sync.dma_start(out=wgi1t[:], in_=wgi1[:, :])
        wgnt = cpool.tile([96, 96], bf)
        nc.sync.dma_start(out=wgnt[:], in_=wgnD[:, :])
        xt1 = bpool.tile([97, DSTS], bf)
        nc.sync.dma_start(out=xt1[:], in_=xt1d[:, :])

        def finish_dummy():
            outsb = bpool.tile([96, DSTS], f32)
            nc.vector.memset(outsb[:], 0.0)
            nc.sync.dma_start(out=outd[:, :], in_=outsb[:])

        swdge_ctr = [0]
        _regs = {}

        def ni_reg(v):
            if v not in _regs:
                _regs[v] = nc.gpsimd.to_reg(v)
            return _regs[v]

        def next_q():
            q = swdge_ctr[0] % NQ
            swdge_ctr[0] += 1
            return q

        def gather_scale(batch_i, slot_off, idx_off, nwb, Db, src_ap):
            """Chunked gather + a-scale; yields (msgs_tile, g0, ng)."""
            out = []
            g0 = 0
            while g0 < Db:
                ng = min(GCH, Db - g0)
                nslots = ng * nwb
                ni = nslots * LANES
                so = slot_off + g0 * nwb
                io = idx_off + g0 * nwb * IW
                msgs = mpool.tile([128, nslots, 128], bf, tag="msgs")
                nc.gpsimd.dma_gather(
                    msgs[:],
                    src_ap,
                    gi[:, io:io + nslots * IW],
                    ni,
                    ni_reg(ni),
                    128,
                    queue_num=next_q(),
                    single_packet=False,
                )
                if phase >= 3:
                    m4 = msgs[:].rearrange("p s (c t) -> p s c t", t=2)[:, :, :49, :]
                    a4 = (
                        a2[:, 2 * so:2 * (so + nslots)]
                        .rearrange("p (s t) -> p s t", t=2)
                        .unsqueeze(2)
                        .broadcast_to((128, nslots, 49, 2))
                    )
                    nc.vector.tensor_tensor(m4, m4, a4, mybir.AluOpType.mult)
                out.append((msgs, g0, ng))
                g0 += ng
            return out

        if phase <= 1:
            finish_dummy()
            return
        if phase <= 3:
            (w0, nwb, Db) = meta["b_hi"][0]
            msgs = gather_scale(0, 0, 0, nwb, Db, xtab[HALF:, :])
            fl = fpool.tile([128, nwb, 128], bf, tag="fl")
            nc.vector.tensor_copy(fl[:], msgs[:, :nwb, :])
            dst = hscr[:nwb * LANES, :].rearrange("(w p) c -> p w c", p=LANES)
            nc.sync.dma_start(out=dst, in_=fl[:])
            finish_dummy()
            return

        # ---- hi pass: aggregate in hi-order, bounce via HBM scratch
        slot_off, idx_off, bi = 0, 0, 0
        for (w0, nwb, Db) in meta["b_hi"]:
            fl = fpool.tile([128, nwb, 128], bf, tag="fl")
            if Db == 0:
                nc.vector.memset(fl[:], 0.0)
            else:
                chunks = gather_scale(bi, slot_off, idx_off, nwb, Db,
                                      xtab[HALF:, :])
                ps = ps_e.tile([128, nwb, 97], f32, tag="pse")
                for (msgs, g0, ng) in chunks:
                    for g in range(ng):
                        nc.tensor.matmul(
                            ps[:], ident[:],
                            msgs[:, g * nwb:(g + 1) * nwb, :97],
                            start=(g0 + g == 0), stop=(g0 + g == Db - 1))
                nc.vector.memset(fl[:, :, 97:], 0.0)
                nc.scalar.activation(fl[:, :, :97], ps[:],
                                     mybir.ActivationFunctionType.Copy)
            dst = hscr[w0 * LANES:(w0 + nwb) * LANES, :].rearrange(
                "(w p) c -> p w c", p=LANES)
            nc.sync.dma_start(out=dst, in_=fl[:])
            slot_off += Db * nwb
            idx_off += Db * nwb * IW
            bi += 1

        if phase <= 4:
            finish_dummy()
            return

        # ---- lo pass: aggregate into persistent agg0lo (lane-major)
        agg0T = bpool.tile([97, DSTS], bf)
        agg0lo = bpool.tile([128, NW, 128], bf)
        for (w0, nwb, Db) in meta["b_lo"]:
            dstsl = agg0lo[:, w0:w0 + nwb, :]
            if Db == 0:
                nc.vector.memset(dstsl, 0.0)
            else:
                ps = ps_e.tile([128, nwb, 97], f32, tag="pse")
                chunks = gather_scale(bi, slot_off, idx_off, nwb, Db,
                                      xtab[:HALF, :])
                for (msgs, g0, ng) in chunks:
                    for g in range(ng):
                        nc.tensor.matmul(
                            ps[:], ident[:],
                            msgs[:, g * nwb:(g + 1) * nwb, :97],
                            start=(g0 + g == 0), stop=(g0 + g == Db - 1))
                nc.scalar.activation(dstsl[:, :, :97], ps[:],
                                     mybir.ActivationFunctionType.Copy)
            slot_off += Db * nwb
            idx_off += Db * nwb * IW
            bi += 1

        # ---- merge permuted hi aggregate, transpose windows into agg0T
        perm_idx0 = (meta["s_hi"] + meta["s_lo"]) * IW
        for (w0, nwb, Db) in meta["b_lo"]:
            permt = mpool.tile([128, nwb, 128], bf, tag="perm")
            ni = nwb * LANES
            nc.gpsimd.dma_gather(
                permt[:],
                hscr[:, :],
                gi[:, perm_idx0 + w0 * (LANES // 16):
                   perm_idx0 + (w0 + nwb) * (LANES // 16)],
                ni,
                ni_reg(ni),
                128,
                queue_num=next_q(),
                single_packet=False,
            )
            mg = agg0lo[:, w0:w0 + nwb, :97]
            nc.vector.tensor_tensor(mg, mg, permt[:, :, :97],
                                    mybir.AluOpType.add)
            for wi in range(nwb):
                tp = ps_t.tile([97, 128], bf, tag="tp")
                nc.tensor.transpose(
                    tp[:], agg0lo[:, w0 + wi, :97], ident[:])
                nc.vector.tensor_copy(
                    agg0T[:, (w0 + wi) * LANES:(w0 + wi + 1) * LANES],
                    tp[:])

        if phase <= 5:
            finish_dummy()
            return

        # ---- fold Wn/bn, gate, combine + store (feature-major 512 tiles)
        aggT = bpool.tile([96, DSTS], bf)
        c0 = 0
        while c0 < DSTS:
            w = min(512, DSTS - c0)
            fp = ps_f.tile([96, w], f32, tag="fg")
            nc.tensor.matmul(fp[:], wn1t[:], agg0T[:, c0:c0 + w],
                             start=True, stop=True)
            nc.scalar.activation(aggT[:, c0:c0 + w], fp[:],
                                 mybir.ActivationFunctionType.Copy)
            gp = ps_f.tile([96, w], f32, tag="fg")
            nc.tensor.matmul(gp[:], wgi1t[:], xt1[:, c0:c0 + w],
                             start=True, stop=False)
            nc.tensor.matmul(gp[:], wgnt[:], aggT[:, c0:c0 + w],
                             start=False, stop=True)
            gate = fpool.tile([96, 512], bf, tag="gate")
            nc.scalar.activation(gate[:, :w], gp[:],
                                 mybir.ActivationFunctionType.Sigmoid)
            diff = fpool.tile([96, 512], bf, tag="diff")
            nc.vector.tensor_tensor(diff[:, :w], aggT[:, c0:c0 + w],
                                    xt1[:96, c0:c0 + w],
                                    mybir.AluOpType.subtract)
            nc.vector.tensor_tensor(diff[:, :w], diff[:, :w], gate[:, :w],
                                    mybir.AluOpType.mult)
            outsb = fpool.tile([96, 512], f32, tag="outsb")
            nc.vector.tensor_tensor(outsb[:, :w], diff[:, :w],
                                    xt1[:96, c0:c0 + w],
                                    mybir.AluOpType.add)
            nc.sync.dma_start(out=outd[:, c0:c0 + w], in_=outsb[:, :w])
            c0 += w

    with tile.TileContext(nc) as tc:
        with (
            tc.tile_pool(name="const", bufs=1) as cpool,
            tc.tile_pool(name="msgs", bufs=14) as mpool,
            tc.tile_pool(name="flush", bufs=4) as fpool,
            tc.tile_pool(name="big", bufs=1) as bpool,
            tc.tile_pool(name="ps_edge", bufs=5, space="PSUM") as ps_e,
            tc.tile_pool(name="ps_tp", bufs=1, space="PSUM") as ps_t,
            tc.tile_pool(name="ps_fg", bufs=2, space="PSUM") as ps_f,
        ):
            emit(tc, cpool, mpool, fpool, bpool, ps_e, ps_t, ps_f)

    nc.compile()

    # Align each gather's SWDGE queue with its Tile-assigned DMASW lane so a
    # semaphore is only ever updated from one queue (ucode shadow-sem rule).
    for ins in nc.inst_map.values():
        if isinstance(ins, mybir.InstDMAGatherAnt):
            si = ins.sync_info
            if si and si.on_update:
                lane = int(si.on_update[0].ant_name.split("_")[0][5:])
                ins.queue_num = lane % NQ
    return nc


# ---------------------------------------------------------------- entrypoint

_CACHE = {}


def kernel(X, a_vals, Wn, bn, Wgi, bgi, Wgn, bgn, row, col):
    _install_ntff_hook()
    from concourse.bass_utils import run_bass_kernel_spmd

    meta, per_core, sig_lo = prep(X, a_vals, row, col)
    key = (str(meta["b_lo"]), str(meta["b_hi"]))
    if key not in _CACHE:
        _CACHE[key] = build(meta)
    nc = _CACHE[key]

    Wn, bn, Wgi, bgi, Wgn, bgn = (np.asarray(w, np.float32)
                                  for w in (Wn, bn, Wgi, bgi, Wgn, bgn))
    wts = dict(
        wn1=np.concatenate([Wn, bn[None]], 0).astype(BF16),
        wgi1=np.concatenate([Wgi, (bgi + bgn)[None]], 0).astype(BF16),
        wgn=Wgn.astype(BF16),
    )
    in_maps = [dict(c, **wts) for c in per_core]
    res = run_bass_kernel_spmd(nc, in_maps, core_ids=list(range(CORES)),
                               trace=kernel._trace)
    kernel._last = res

    out = np.empty((N, 96), np.float32)
    for k in range(CORES):
        o = res.results[k]["out"]  # [96, DSTS]
        sig = sig_lo[k]
        realm = sig < DPC
        out[k * DPC + sig[realm]] = o[:, realm].T
    return out


kernel._trace = False
kernel._last = None


# revision 30
# speedup vs baseline: 1.0932x; 1.0482x over previous
"""GatedGCN message-passing layer on 8 Trainium2 NeuronCores.

Strategy (dst-sharded, aggregate-first):
  - Node rows (dsts) sharded across 8 cores (6250 each).
  - agg0 = sum_e a_e * X[col_e]  computed on-device: X table (bf16, 256B rows)
    lives in HBM; edges gathered with dma_gather; per-edge a-scale on DVE;
    segment-sum via identity-matmul PSUM accumulation over degree-sorted
    128-dst windows (one edge slot per dst lane per group).
  - agg = agg0 @ Wn + (sum_e a_e) * bn   (linearity: fold Wn after aggregation)
  - gate = sigmoid(X@Wgi + bgi + agg@Wgn + bgn); out = gate*(agg-X)+X.
  - dma_gather indices are int16 -> X table split in two 25000-row halves; the
    hi-half aggregate is computed in its own degree-sorted order, bounced
    through an HBM scratch and re-gathered into lo-order for the merge.
"""

import sys
import types

sys.path.insert(0, "/opt/trn_rl_repo")

import numpy as np
import ml_dtypes

BF16 = ml_dtypes.bfloat16

N = 50000
E = 800000
D = 96
CORES = 8
DPC = N // CORES  # dsts per core
HALF = N // 2  # table split for int16 indices
LANES = 128
WB = 5  # windows per matmul batch (5*97=485 <= 512 psum free)
NW = (DPC + LANES - 1) // LANES  # 49 windows per pass
NQ = 4  # SWDGE queues in use
GCH = 3  # groups per gather chunk
DSTS = NW * LANES  # 6272 padded dst slots per core


def _install_ntff_hook():
    try:
        import antenv.axon_hooks  # noqa: F401

        return
    except ImportError:
        pass
    try:
        from trn_agent_boot.trn_boot import _ntff_profile_via_ctypes

        hook = _ntff_profile_via_ctypes("/opt/axon/libaxon_pjrt.so")
        mod = types.ModuleType("antenv.axon_hooks")
        mod.get_axon_ntff_profile_hook = lambda: hook
        mod.set_axon_ntff_profile_hook = lambda h: None
        sys.modules["antenv.axon_hooks"] = mod
    except Exception:
        pass


# ---------------------------------------------------------------- host prep


def _prep_pass(ldst, tloc, av, nw):
    """Per-pass structures for one core.

    ldst: local dst id per edge; tloc: table-local src row; av: a value.
    Returns dict with sigma (dst order), deg (per local dst), and per-dst
    sorted edge arrays (tloc/av ordered by dst, with offsets).
    """
    dsts = nw * LANES
    deg = np.bincount(ldst, minlength=dsts).astype(np.int64)
    order = np.argsort(ldst, kind="stable")
    offs = np.zeros(dsts + 1, np.int64)
    np.cumsum(deg, out=offs[1:])
    sigma = np.argsort(-deg, kind="stable")
    return dict(
        deg=deg,
        offs=offs,
        tloc_sorted=tloc[order],
        av_sorted=av[order],
        sigma=sigma,
        wmax=deg[sigma[::LANES]].copy(),  # max degree per window
    )


def _batches(nw, wmax_shared):
    """Static batch structure shared by all cores: list of (w0, nwb, Db)."""
    out = []
    for b in range((nw + WB - 1) // WB):
        w0 = b * WB
        nwb = min(WB, nw - w0)
        Db = int(wmax_shared[w0:w0 + nwb].max()) if nwb else 0
        out.append((w0, nwb, Db))
    return out


def _fill_pass(p, batches, slot_off, idx_off, gidx16, a2, nw):
    """Fill gather-index and a-value arrays for one pass of one core."""
    sigma, deg, offs = p["sigma"], p["deg"], p["offs"]
    tls, avs = p["tloc_sorted"], p["av_sorted"]
    s = slot_off
    for (w0, nwb, Db) in batches:
        if Db == 0:
            continue
        # dstm[g, wi, l] = local dst at window w0+wi lane l
        lanes = sigma[w0 * LANES:(w0 + nwb) * LANES].reshape(nwb, LANES)
        dstm = np.broadcast_to(lanes[None, :, :], (Db, nwb, LANES))
        g = np.arange(Db)[:, None, None]
        valid = g < deg[dstm]
        eidx = offs[dstm] + np.minimum(g, np.maximum(deg[dstm] - 1, 0))
        eidx = np.minimum(eidx, max(len(tls) - 1, 0))
        iv = np.where(valid, tls[eidx], 0).astype(np.int16)
        avv = np.where(valid, avs[eidx], 0.0).astype(np.float32)
        nslots = Db * nwb
        # position n = (slot-local)*128 + lane ; slot-local = g*nwb+wi
        ivf = iv.reshape(nslots * LANES)
        avf = avv.reshape(nslots, LANES)
        # index layout: position n -> [n%16, idx_off + n//16]
        c0 = idx_off + s * (LANES // 16) - slot_off * (LANES // 16)
        cols = nslots * LANES // 16
        gidx16[:, c0:c0 + cols] = ivf.reshape(cols, 16).T
        a2[:, 2 * s:2 * (s + nslots):2] = avf.T
        a2[:, 2 * s + 1:2 * (s + nslots) + 1:2] = avf.T
        s += nslots
    return s


def prep(X, a_vals, row, col):
    """Returns (meta, per_core_inputs, host_ctx)."""
    row = np.asarray(row).astype(np.int64)
    col = np.asarray(col).astype(np.int64)
    av = np.asarray(a_vals, np.float32)
    X = np.asarray(X, np.float32)

    # shared bf16 X table, 128-col rows, col 96 = 1.0 (for sum-of-a column)
    xtab = np.zeros((N, 128), BF16)
    xtab[:, :96] = X.astype(BF16)
    xtab[:, 96] = BF16(1.0)

    core = row // DPC
    passes = []  # per core: (lo_pass, hi_pass)
    for k in range(CORES):
        m = core == k
        rk, ck, ak = row[m] - k * DPC, col[m], av[m]
        mlo = ck < HALF
        lo = _prep_pass(rk[mlo], ck[mlo], ak[mlo], NW)
        hi = _prep_pass(rk[~mlo], ck[~mlo] - HALF, ak[~mlo], NW)
        passes.append((lo, hi))

    # shared (max over cores) window-degree profiles -> static batch structure
    wmax_lo = np.max([p[0]["wmax"] for p in passes], axis=0)
    wmax_hi = np.max([p[1]["wmax"] for p in passes], axis=0)
    b_lo = _batches(NW, wmax_lo)
    b_hi = _batches(NW, wmax_hi)
    s_hi = sum(d * n for (_, n, d) in b_hi)
    s_lo = sum(d * n for (_, n, d) in b_lo)
    n_perm = DSTS  # one gathered row per lo-ordered dst slot
    idx_cols = (s_hi + s_lo) * (LANES // 16) + n_perm // 16
    meta = dict(b_lo=b_lo, b_hi=b_hi, s_hi=s_hi, s_lo=s_lo,
                idx_cols=idx_cols, a2_cols=2 * (s_hi + s_lo))

    per_core = []
    sig_lo_all = []
    for k in range(CORES):
        lo, hi = passes[k]
        gidx16 = np.zeros((16, idx_cols), np.int16)
        a2 = np.zeros((128, meta["a2_cols"]), BF16)
        nh = _fill_pass(hi, b_hi, 0, 0, gidx16, a2, NW)
        assert nh == s_hi
        nl = _fill_pass(lo, b_lo, s_hi, s_hi * (LANES // 16), gidx16, a2, NW)
        assert nl == s_hi + s_lo
        # perm indices: lo-order position j -> hi-order position of same dst
        pos_hi = np.empty(DSTS, np.int64)
        pos_hi[hi["sigma"]] = np.arange(DSTS)
        pidx = pos_hi[lo["sigma"]].astype(np.int16)
        c0 = (s_hi + s_lo) * (LANES // 16)
        gidx16[:, c0:] = pidx.reshape(DSTS // 16, 16).T
        gidx = np.tile(gidx16, (8, 1))

        # per-core node features, lo-order, feature-major, with ones row
        gids = np.minimum(k * DPC + lo["sigma"], N - 1)
        realm = lo["sigma"] < DPC
        xs = np.where(realm[:, None], X[gids], 0.0)
        xt1s = np.zeros((97, DSTS), BF16)
        xt1s[:96] = xs.T.astype(BF16)
        xt1s[96] = np.where(realm, 1.0, 0.0).astype(BF16)
        per_core.append(dict(xtab=xtab, gidx=gidx, a2=a2, xt1s=xt1s))
        sig_lo_all.append(lo["sigma"])

    return meta, per_core, sig_lo_all


# ---------------------------------------------------------------- bass build


def build(meta, phase=99):
    import concourse.bacc as bacc
    import concourse.mybir as mybir
    import concourse.tile as tile

    bf = mybir.dt.bfloat16
    f32 = mybir.dt.float32
    nc = bacc.Bacc(None, target_bir_lowering=False, num_swdge_queues=4)

    xtab = nc.dram_tensor("xtab", [N, 128], bf, kind="ExternalInput")
    gidx = nc.dram_tensor("gidx", [128, meta["idx_cols"]], mybir.dt.int16,
                          kind="ExternalInput")
    a2d = nc.dram_tensor("a2", [128, meta["a2_cols"]], bf, kind="ExternalInput")
    xt1d = nc.dram_tensor("xt1s", [97, DSTS], bf, kind="ExternalInput")
    wn1 = nc.dram_tensor("wn1", [97, 96], bf, kind="ExternalInput")
    wgi1 = nc.dram_tensor("wgi1", [97, 96], bf, kind="ExternalInput")
    wgnD = nc.dram_tensor("wgn", [96, 96], bf, kind="ExternalInput")
    outd = nc.dram_tensor("out", [96, DSTS], f32, kind="ExternalOutput")
    hscr = nc.dram_tensor("hscr", [DSTS, 128], bf, kind="Internal")

    identD = nc.inline_tensor(np.eye(128, dtype=BF16), "ident")

    IW = LANES // 16  # idx cols per slot

    def emit(tc, cpool, mpool, fpool, bpool, ps_e, ps_t, ps_f):
        gi = bpool.tile([128, meta["idx_cols"]], mybir.dt.int16)
        nc.sync.dma_start(out=gi[:], in_=gidx[:, :])
        ident = cpool.tile([128, 128], bf)
        nc.sync.dma_start(out=ident[:], in_=identD[:, :])
        a2 = bpool.tile([128, meta["a2_cols"]], bf)
        nc.sync.dma_start(out=a2[:], in_=a2d[:, :])
        wn1t = cpool.tile([97, 96], bf)
        nc.sync.dma_start(out=wn1t[:], in_=wn1[:, :])
        wgi1t = cpool.tile([97, 96], bf)
        nc.sync.dma_start(out=wgi1t[:], in_=wgi1[:, :])
        wgnt = cpool.tile([96, 96], bf)
        nc.sync.dma_start(out=wgnt[:], in_=wgnD[:, :])
        xt1 = bpool.tile([97, DSTS], bf)
        nc.sync.dma_start(out=xt1[:], in_=xt1d[:, :])

        def finish_dummy():
            outsb = bpool.tile([96, DSTS], f32)
            nc.vector.memset(outsb[:], 0.0)
            nc.sync.dma_start(out=outd[:, :], in_=outsb[:])

        swdge_ctr = [0]
        _regs = {}

        def ni_reg(v):
            if v not in _regs:
                _regs[v] = nc.gpsimd.to_reg(v)
            return _regs[v]

        def next_q():
            q = swdge_ctr[0] % NQ
            swdge_ctr[0] += 1
            return q

        def gather_scale(batch_i, slot_off, idx_off, nwb, Db, src_ap):
            """Chunked gather + a-scale; yields (msgs_tile, g0, ng)."""
            out = []
            g0 = 0
            while g0 < Db:
                ng = min(GCH, Db - g0)
                nslots = ng * nwb
                ni = nslots * LANES
                so = slot_off + g0 * nwb
                io = idx_off + g0 * nwb * IW
                msgs = mpool.tile([128, nslots, 128], bf, tag="msgs")
                nc.gpsimd.dma_gather(
                    msgs[:],
                    src_ap,
                    gi[:, io:io + nslots * IW],
                    ni,
                    ni_reg(ni),
                    128,
                    queue_num=next_q(),
                    single_packet=False,
                )
                if phase >= 3:
                    m4 = msgs[:].rearrange("p s (c t) -> p s c t", t=2)[:, :, :49, :]
                    a4 = (
                        a2[:, 2 * so:2 * (so + nslots)]
                        .rearrange("p (s t) -> p s t", t=2)
                        .unsqueeze(2)
                        .broadcast_to((128, nslots, 49, 2))
                    )
                    nc.vector.tensor_tensor(m4, m4, a4, mybir.AluOpType.mult)
                out.append((msgs, g0, ng))
                g0 += ng
            return out

        if phase <= 1:
            finish_dummy()
            return

        # Global group stream per pass; uniform GCH-group gather chunks
        # that may cross batch boundaries.
        def run_pass(batches, slot_off0, idx_off0, src_ap, sink):
            """sink(batch_idx, w0, nwb, ps) called when a batch's psum closes."""
            groups = []  # (batch_i, w0, nwb, g, Db, slot_off)
            s = slot_off0
            for bi_, (w0, nwb, Db) in enumerate(batches):
                for g in range(Db):
                    groups.append((bi_, w0, nwb, g, Db, s))
                    s += nwb
            psums = {}
            ci = 0
            while ci < len(groups):
                chunk = groups[ci:ci + GCH]
                ci += len(chunk)
                c_slot0 = chunk[0][5]
                nslots = sum(c[2] for c in chunk)
                ni = nslots * LANES
                io = idx_off0 + (c_slot0 - slot_off0) * IW
                msgs = mpool.tile([128, nslots, 128], bf, tag="msgs")
                nc.gpsimd.dma_gather(
                    msgs[:],
                    src_ap,
                    gi[:, io:io + nslots * IW],
                    ni,
                    ni_reg(ni),
                    128,
                    queue_num=next_q(),
                    single_packet=False,
                )
                if phase >= 3:
                    m4 = msgs[:].rearrange(
                        "p s (c t) -> p s c t", t=2)[:, :, :49, :]
                    a4 = (
                        a2[:, 2 * c_slot0:2 * (c_slot0 + nslots)]
                        .rearrange("p (s t) -> p s t", t=2)
                        .unsqueeze(2)
                        .broadcast_to((128, nslots, 49, 2))
                    )
                    nc.vector.tensor_tensor(m4, m4, a4, mybir.AluOpType.mult)
                for (bi_, w0, nwb, g, Db, so) in chunk:
                    lo_s = so - c_slot0
                    if bi_ not in psums:
                        pstile = ps_e.tile([128, nwb, 97], f32, tag="pse")
                        psums[bi_] = pstile
                    nc.tensor.matmul(
                        psums[bi_][:], ident[:],
                        msgs[:, lo_s:lo_s + nwb, :97],
                        start=(g == 0), stop=(g == Db - 1))
                    if g == Db - 1:
                        sink(bi_, w0, nwb, psums.pop(bi_))

        # ---- hi pass: aggregate in hi-order, bounce via HBM scratch
        def hi_sink(bi_, w0, nwb, ps):
            fl = fpool.tile([128, nwb, 128], bf, tag="fl")
            nc.vector.memset(fl[:, :, 97:], 0.0)
            nc.scalar.activation(fl[:, :, :97], ps[:],
                                 mybir.ActivationFunctionType.Copy)
            dst = hscr[w0 * LANES:(w0 + nwb) * LANES, :].rearrange(
                "(w p) c -> p w c", p=LANES)
            nc.sync.dma_start(out=dst, in_=fl[:])

        for (w0, nwb, Db) in meta["b_hi"]:
            if Db == 0:
                fl = fpool.tile([128, nwb, 128], bf, tag="fl")
                nc.vector.memset(fl[:], 0.0)
                dst = hscr[w0 * LANES:(w0 + nwb) * LANES, :].rearrange(
                    "(w p) c -> p w c", p=LANES)
                nc.sync.dma_start(out=dst, in_=fl[:])
        run_pass(meta["b_hi"], 0, 0, xtab[HALF:, :], hi_sink)

        if phase <= 4:
            finish_dummy()
            return

        # ---- lo pass into persistent agg0lo (lane-major)
        agg0T = bpool.tile([97, DSTS], bf)
        agg0lo = bpool.tile([128, NW, 128], bf)

        def lo_sink(bi_, w0, nwb, ps):
            nc.scalar.activation(agg0lo[:, w0:w0 + nwb, :97], ps[:],
                                 mybir.ActivationFunctionType.Copy)

        for (w0, nwb, Db) in meta["b_lo"]:
            if Db == 0:
                nc.vector.memset(agg0lo[:, w0:w0 + nwb, :], 0.0)
        run_pass(meta["b_lo"], meta["s_hi"], meta["s_hi"] * IW,
                 xtab[:HALF, :], lo_sink)

        # ---- merge permuted hi aggregate, transpose windows into agg0T
        perm_idx0 = (meta["s_hi"] + meta["s_lo"]) * IW
        for (w0, nwb, Db) in meta["b_lo"]:
            permt = mpool.tile([128, nwb, 128], bf, tag="perm")
            ni = nwb * LANES
            nc.gpsimd.dma_gather(
                permt[:],
                hscr[:, :],
                gi[:, perm_idx0 + w0 * (LANES // 16):
                   perm_idx0 + (w0 + nwb) * (LANES // 16)],
                ni,
                ni_reg(ni),
                128,
                queue_num=next_q(),
                single_packet=False,
            )
            mg = agg0lo[:, w0:w0 + nwb, :97]
            nc.vector.tensor_tensor(mg, mg, permt[:, :, :97],
                                    mybir.AluOpType.add)
            for wi in range(nwb):
                tp = ps_t.tile([97, 128], bf, tag="tp")
                nc.tensor.transpose(
                    tp[:], agg0lo[:, w0 + wi, :97], ident[:])
                nc.vector.tensor_copy(
                    agg0T[:, (w0 + wi) * LANES:(w0 + wi + 1) * LANES],
                    tp[:])

        # ---- fold Wn/bn, gate, combine + store (feature-major 512 tiles)
        aggT = bpool.tile([96, DSTS], bf)
        c0 = 0
        while c0 < DSTS:
            w = min(512, DSTS - c0)
            fp = ps_f.tile([96, w], f32, tag="fg")
            nc.tensor.matmul(fp[:], wn1t[:], agg0T[:, c0:c0 + w],
                             start=True, stop=True)
            nc.scalar.activation(aggT[:, c0:c0 + w], fp[:],
                                 mybir.ActivationFunctionType.Copy)
            gp = ps_f.tile([96, w], f32, tag="fg")
            nc.tensor.matmul(gp[:], wgi1t[:], xt1[:, c0:c0 + w],
                             start=True, stop=False)
            nc.tensor.matmul(gp[:], wgnt[:], aggT[:, c0:c0 + w],
                             start=False, stop=True)
            gate = fpool.tile([96, 512], bf, tag="gate")
            nc.scalar.activation(gate[:, :w], gp[:],
                                 mybir.ActivationFunctionType.Sigmoid)
            diff = fpool.tile([96, 512], bf, tag="diff")
            nc.vector.tensor_tensor(diff[:, :w], aggT[:, c0:c0 + w],
                                    xt1[:96, c0:c0 + w],
                                    mybir.AluOpType.subtract)
            nc.vector.tensor_tensor(diff[:, :w], diff[:, :w], gate[:, :w],
                                    mybir.AluOpType.mult)
            outsb = fpool.tile([96, 512], f32, tag="outsb")
            nc.vector.tensor_tensor(outsb[:, :w], diff[:, :w],
                                    xt1[:96, c0:c0 + w],
                                    mybir.AluOpType.add)
            nc.sync.dma_start(out=outd[:, c0:c0 + w], in_=outsb[:, :w])
            c0 += w

    with tile.TileContext(nc) as tc:
        with (
            tc.tile_pool(name="const", bufs=1) as cpool,
            tc.tile_pool(name="msgs", bufs=14) as mpool,
            tc.tile_pool(name="flush", bufs=4) as fpool,
            tc.tile_pool(name="big", bufs=1) as bpool,
            tc.tile_pool(name="ps_edge", bufs=5, space="PSUM") as ps_e,
            tc.tile_pool(name="ps_tp", bufs=1, space="PSUM") as ps_t,
            tc.tile_pool(name="ps_fg", bufs=2, space="PSUM") as ps_f,
        ):
            emit(tc, cpool, mpool, fpool, bpool, ps_e, ps_t, ps_f)

    nc.compile()

    # Align each gather's SWDGE queue with its Tile-assigned DMASW lane so a
    # semaphore is only ever updated from one queue (ucode shadow-sem rule).
    for ins in nc.inst_map.values():
        if isinstance(ins, mybir.InstDMAGatherAnt):
            si = ins.sync_info
            if si and si.on_update:
                lane = int(si.on_update[0].ant_name.split("_")[0][5:])
                ins.queue_num = lane % NQ
    return nc


# ---------------------------------------------------------------- entrypoint

_CACHE = {}


def kernel(X, a_vals, Wn, bn, Wgi, bgi, Wgn, bgn, row, col):
    _install_ntff_hook()
    from concourse.bass_utils import run_bass_kernel_spmd

    meta, per_core, sig_lo = prep(X, a_vals, row, col)
    key = (str(meta["b_lo"]), str(meta["b_hi"]))
    if key not in _CACHE:
        _CACHE[key] = build(meta)
    nc = _CACHE[key]

    Wn, bn, Wgi, bgi, Wgn, bgn = (np.asarray(w, np.float32)
                                  for w in (Wn, bn, Wgi, bgi, Wgn, bgn))
    wts = dict(
        wn1=np.concatenate([Wn, bn[None]], 0).astype(BF16),
        wgi1=np.concatenate([Wgi, (bgi + bgn)[None]], 0).astype(BF16),
        wgn=Wgn.astype(BF16),
    )
    in_maps = [dict(c, **wts) for c in per_core]
    res = run_bass_kernel_spmd(nc, in_maps, core_ids=list(range(CORES)),
                               trace=kernel._trace)
    kernel._last = res

    out = np.empty((N, 96), np.float32)
    for k in range(CORES):
        o = res.results[k]["out"]  # [96, DSTS]
        sig = sig_lo[k]
        realm = sig < DPC
        out[k * DPC + sig[realm]] = o[:, realm].T
    return out


kernel._trace = False
kernel._last = None
